# revision 1
# baseline (speedup 1.0000x reference)
"""Trainium2 Bass kernel for the MixedGNN problem (GCN -> GAT -> SAGE -> linear+log_softmax).

Sharding: nodes are permuted into 128-node blocks balanced by in-degree; each of the
8 cores owns a contiguous range of blocks (its slab). Edges live with their
destination block (self loops are explicit edges for GCN/GAT, disabled for SAGE).
Per-edge source rows are fetched with dma_gather (int16 indices, so gather tables
are split into two halves and each block's edges are grouped by source half).
Segment sums are one-hot matmuls accumulating in PSUM; GCN normalization and the
GAT softmax are applied per node, not per edge. Node tables needed by later layers
are exchanged with AllGather.

Host-side work is integer-only packing/permutation metadata; all floating-point
model math runs on the NeuronCores.
"""

import os
import sys
import heapq

import numpy as np

sys.path.insert(0, "/opt/trn_rl_repo")

import concourse.tile as tile  # noqa: E402
from concourse import bacc, mybir  # noqa: E402
from concourse.bass_utils import run_bass_kernel_spmd  # noqa: E402

F32 = mybir.dt.float32
BF16 = mybir.dt.bfloat16
I16 = mybir.dt.int16
ALU = mybir.AluOpType
ACTF = mybir.ActivationFunctionType

NC = 8
P = 128
D_IN = 128
D_H = 128
H = 2
D_OUT = 32
NEG_SLOPE = 0.2
TBLW = 320  # hw-table row stride in f32 (256 hw | 2 a_s | pad) -- 1280B, %256==0


# ----------------------------------------------------------------------------
# Host packing (integer only)
# ----------------------------------------------------------------------------

def _assign_blocks(w, nblk, rng):
    """Greedy balanced assignment of nodes to blocks (<=128 nodes each)."""
    n = len(w)
    order = np.lexsort((rng.permutation(n), -w))
    blk_of = np.empty(n, np.int32)
    heap = [(0, b) for b in range(nblk)]
    heapq.heapify(heap)
    nodecnt = np.zeros(nblk, np.int32)
    for i in order:
        load, b = heapq.heappop(heap)
        blk_of[i] = b
        nodecnt[b] += 1
        if nodecnt[b] < P:
            heapq.heappush(heap, (load + int(w[i]), b))
    return blk_of


def _pack(edge_index, N):
    E = edge_index.shape[1]
    src = np.asarray(edge_index[0], dtype=np.int64)
    dst = np.asarray(edge_index[1], dtype=np.int64)
    NBLK = NC * int(np.ceil(N / (P * NC)))
    NPAD = NBLK * P
    HALF = NPAD // 2
    BPC = NBLK // NC
    SLAB = BPC * P

    deg_in = np.bincount(dst, minlength=N).astype(np.int64)
    w = deg_in + 1  # incoming edges incl. self loop

    best = None
    rng = np.random.default_rng(1234)
    for _try in range(6):
        blk_of = _assign_blocks(w, NBLK, rng)
        order = np.argsort(blk_of, kind="stable")
        cnt = np.bincount(blk_of, minlength=NBLK)
        starts = np.zeros(NBLK + 1, np.int64)
        np.cumsum(cnt, out=starts[1:])
        slot = np.arange(N) - starts[blk_of[order]]
        perm = np.empty(N, np.int64)
        perm[order] = blk_of[order] * P + slot
        esrc = np.concatenate([src, np.arange(N)])
        edst = np.concatenate([dst, np.arange(N)])
        psrc = perm[esrc]
        pdst = perm[edst]
        key = (pdst >> 7) * 2 + (psrc >= HALF)
        counts = np.bincount(key, minlength=NBLK * 2)
        t_half = int(np.ceil(counts.max() / P))
        if best is None or t_half < best[0]:
            best = (t_half, perm, psrc, pdst, counts)
        if t_half <= max(2, int(np.ceil(counts.mean() / P))):
            break
    t_half, perm, psrc, pdst, counts = best
    T = 2 * t_half
    SLOT = t_half * P

    esrc = np.concatenate([src, np.arange(N)])
    is_self = np.concatenate([np.zeros(E, bool), np.ones(N, bool)])
    key = (pdst >> 7) * 2 + (psrc >= HALF)
    ordr = np.lexsort((psrc, key))
    ks = key[ordr]
    grp_start = np.concatenate(([0], np.cumsum(counts)))[ks]
    pos_in_grp = np.arange(len(ks)) - grp_start
    slot_pos = ks * SLOT + pos_in_grp

    tot = NBLK * 2 * SLOT
    eidx = np.zeros(tot, np.int64)
    edl = np.full(tot, -1.0, np.float32)
    edeg = np.ones(tot, np.float32)
    esg = np.full(tot, -1.0, np.float32)
    eidx[slot_pos] = psrc[ordr] - (ks % 2) * HALF
    edl[slot_pos] = (pdst[ordr] & 127).astype(np.float32)
    edeg[slot_pos] = w[esrc[ordr]].astype(np.float32)
    esg[slot_pos] = np.where(is_self[ordr], -1.0, (pdst[ordr] & 127).astype(np.float32))

    assert eidx.max() < HALF and eidx.min() >= 0
    eidx16 = eidx.astype(np.int16)

    # idx tiles: flat i -> [i%16, i//16], replicated x8 down partitions
    A = eidx16.reshape(NBLK, 2, SLOT // 16, 16).transpose(0, 1, 3, 2)
    idx_full = np.ascontiguousarray(np.tile(A, (1, 1, 8, 1)))

    edl_r = edl.reshape(NBLK, T, P).transpose(0, 2, 1)
    edeg_r = edeg.reshape(NBLK, T, P).transpose(0, 2, 1)
    esg_r = esg.reshape(NBLK, T, P).transpose(0, 2, 1)
    meta = np.ascontiguousarray(
        np.concatenate([edl_r, edeg_r, esg_r], axis=2).astype(np.float32))
    metaT = np.ascontiguousarray(edl.reshape(NBLK, T * P).astype(np.float32))

    w_p = np.ones(NPAD, np.float32)
    w_p[perm] = w.astype(np.float32)
    sg_p = np.ones(NPAD, np.float32)
    sg_p[perm] = np.maximum(deg_in, 1).astype(np.float32)
    degs = np.ascontiguousarray(
        np.stack([w_p.reshape(NBLK, P), sg_p.reshape(NBLK, P)], axis=2))

    return dict(
        NBLK=NBLK, NPAD=NPAD, HALF=HALF, BPC=BPC, SLAB=SLAB,
        T_half=t_half, T=T, perm=perm,
        idx=idx_full, meta=meta, metaT=metaT, degs=degs,
    )


# ----------------------------------------------------------------------------
# Device program
# ----------------------------------------------------------------------------

def _build_program(pk):
    BPC, T, Th, NPAD, HALF, SLAB = (
        pk["BPC"], pk["T"], pk["T_half"], pk["NPAD"], pk["HALF"], pk["SLAB"])
    NI = Th * P  # idxs per gather

    nc = bacc.Bacc("TRN2", target_bir_lowering=False, num_devices=NC,
                   num_swdge_queues=4, dynamic_dma_scratch_size=65536)

    x_perm = nc.dram_tensor("x_perm", [NPAD, D_IN], F32, kind="ExternalInput")
    idx_d = nc.dram_tensor("idx", [BPC, 2, P, NI // 16], I16, kind="ExternalInput")
    meta_d = nc.dram_tensor("meta", [BPC, P, 3 * T], F32, kind="ExternalInput")
    metaT_d = nc.dram_tensor("metaT", [BPC, T * P], F32, kind="ExternalInput")
    degs_d = nc.dram_tensor("degs", [BPC, P, 2], F32, kind="ExternalInput")
    w_gcn_d = nc.dram_tensor("w_gcn", [D_IN, D_H], F32, kind="ExternalInput")
    w_gat_d = nc.dram_tensor("w_gat", [D_H, H * D_H], F32, kind="ExternalInput")
    att_s_d = nc.dram_tensor("att_s", [P, H * D_H], F32, kind="ExternalInput")
    att_d_d = nc.dram_tensor("att_d", [P, H * D_H], F32, kind="ExternalInput")
    w_sl_d = nc.dram_tensor("w_sl", [D_H, D_H], F32, kind="ExternalInput")
    w_sr_d = nc.dram_tensor("w_sr", [D_H, D_H], F32, kind="ExternalInput")
    w_out_d = nc.dram_tensor("w_out", [D_H, D_OUT], F32, kind="ExternalInput")
    ident_d = nc.dram_tensor("ident", [P, P], F32, kind="ExternalInput")
    iotar_d = nc.dram_tensor("iotar", [P, P], F32, kind="ExternalInput")
    iotac_d = nc.dram_tensor("iotac", [P, 1], F32, kind="ExternalInput")
    onesr_d = nc.dram_tensor("onesr", [1, P], F32, kind="ExternalInput")
    out_d = nc.dram_tensor("out", [SLAB, D_OUT], F32, kind="ExternalOutput")

    rg = [list(range(NC))]
    qn = [0]

    def next_q():
        qn[0] = (qn[0] + 1) % 4
        return qn[0]

    with tile.TileContext(nc) as tc:
        with (
            tc.tile_pool(name="const", bufs=1) as cp,
            tc.tile_pool(name="dram", bufs=1, space="DRAM") as dp,
        ):
            def cload(shape, dt, src, tag):
                t = cp.tile(shape, dt, tag=tag)
                nc.sync.dma_start(out=t[:], in_=src)
                return t

            w_gcn = cload([D_IN, D_H], F32, w_gcn_d[:], "c_wgcn")
            w_gat = cload([D_H, H * D_H], F32, w_gat_d[:], "c_wgat")
            att_s = cload([P, H * D_H], F32, att_s_d[:], "c_atts")
            att_dt = cload([P, H * D_H], F32, att_d_d[:], "c_attd")
            w_sl = cload([D_H, D_H], F32, w_sl_d[:], "c_wsl")
            w_sr = cload([D_H, D_H], F32, w_sr_d[:], "c_wsr")
            w_out = cload([D_H, D_OUT], F32, w_out_d[:], "c_wout")
            ident = cload([P, P], F32, ident_d[:], "c_ident")
            iotar = cload([P, P], F32, iotar_d[:], "c_iotar")
            iotac = cload([P, 1], F32, iotac_d[:], "c_iotac")
            onesr = cload([1, P], F32, onesr_d[:], "c_onesr")

            meta_res = cp.tile([P, BPC * 3 * T], F32)
            for b in range(BPC):
                nc.sync.dma_start(out=meta_res[:, b * 3 * T:(b + 1) * 3 * T],
                                  in_=meta_d[b])
            degs_res = cp.tile([P, BPC * 2], F32)
            for b in range(BPC):
                nc.sync.dma_start(out=degs_res[:, b * 2:(b + 1) * 2], in_=degs_d[b])

            h1_sb = cp.tile([P, SLAB], F32)   # GCN output slab (reused for h3)
            h2_sb = cp.tile([P, SLAB], F32)   # GAT output slab
            ad_sb = cp.tile([P, 2 * BPC], BF16)  # per-own-node a_d

            hwt_slab = dp.tile([SLAB, TBLW], F32)
            hwt_full = dp.tile([NPAD, TBLW], F32)
            h2_slab = dp.tile([SLAB, D_H], F32)
            h2_full = dp.tile([NPAD, D_H], F32)

            def mcol(b, t):
                return meta_res[:, b * 3 * T + t:b * 3 * T + t + 1]

            def mdeg_cols(b):
                return meta_res[:, b * 3 * T + T:b * 3 * T + 2 * T]

            def msage(b, t):
                return meta_res[:, b * 3 * T + 2 * T + t:b * 3 * T + 2 * T + t + 1]

            # =============== Layer 1: GCN (+ hw table build) ===============
            with (
                tc.tile_pool(name="l1w", bufs=3) as wp,
                tc.tile_pool(name="l1p", bufs=2, space="PSUM") as pp,
                tc.tile_pool(name="l1pt", bufs=1, space="PSUM") as ppt,
                tc.tile_pool(name="l1ph", bufs=1, space="PSUM") as pph,
            ):
                for b in range(BPC):
                    g0 = wp.tile([P, Th * D_IN], F32, tag="g1a")
                    g1 = wp.tile([P, Th * D_IN], F32, tag="g1b")
                    for h, g in ((0, g0), (1, g1)):
                        src_ap = x_perm[:] if h == 0 else x_perm[HALF:, :]
                        ixt = wp.tile([P, NI // 16], I16, tag="ix1")
                        nc.sync.dma_start(out=ixt[:], in_=idx_d[b, h])
                        nc.gpsimd.dma_gather(
                            out_ap=g[:].rearrange("p (t w) -> p t w", w=D_IN),
                            in_ap=src_ap,
                            idxs_ap=ixt[:],
                            num_idxs=NI, num_idxs_reg=NI, elem_size=D_IN,
                            single_packet=False, queue_num=next_q())
                    dinv = wp.tile([P, T], F32, tag="dinv")
                    nc.scalar.activation(out=dinv[:], in_=mdeg_cols(b), func=ACTF.Sqrt)
                    nc.vector.reciprocal(out=dinv[:], in_=dinv[:])
                    psum = pp.tile([P, D_H], F32, tag="pg")
                    for t in range(T):
                        h, tr = divmod(t, Th)
                        g = g0 if h == 0 else g1
                        oh = wp.tile([P, P], F32, tag="oh1")
                        nc.vector.tensor_scalar(
                            out=oh[:], in0=iotar[:], scalar1=mcol(b, t),
                            scalar2=dinv[:, t:t + 1], op0=ALU.is_equal, op1=ALU.mult)
                        nc.tensor.matmul(
                            out=psum[:], lhsT=oh[:],
                            rhs=g[:, tr * D_IN:(tr + 1) * D_IN],
                            start=(t == 0), stop=(t == T - 1))
                    # epilogue: h1 = relu((dinv_i * psum) @ W_gcn)
                    dinv_o = wp.tile([P, 1], F32, tag="dv0")
                    nc.scalar.activation(out=dinv_o[:],
                                         in_=degs_res[:, 2 * b:2 * b + 1],
                                         func=ACTF.Sqrt)
                    nc.vector.reciprocal(out=dinv_o[:], in_=dinv_o[:])
                    pre = wp.tile([P, D_IN], F32, tag="pre")
                    nc.vector.tensor_scalar(out=pre[:], in0=psum[:], scalar1=dinv_o[:],
                                            scalar2=None, op0=ALU.mult)
                    tps0 = ppt.tile([P, P], F32, tag="tr1")
                    nc.tensor.transpose(out=tps0[:], in_=pre[:], identity=ident[:])
                    preT = wp.tile([P, P], F32, tag="preT")
                    nc.vector.tensor_copy(out=preT[:], in_=tps0[:])
                    gcn_ps = pph.tile([P, D_H], F32, tag="gc")
                    nc.tensor.matmul(out=gcn_ps[:], lhsT=preT[:], rhs=w_gcn[:],
                                     start=True, stop=True)
                    h1_blk = h1_sb[:, b * P:(b + 1) * P]
                    nc.scalar.activation(out=h1_blk, in_=gcn_ps[:], func=ACTF.Relu)
                    # hw table build
                    tps = ppt.tile([P, P], F32, tag="tr1")
                    nc.tensor.transpose(out=tps[:], in_=h1_blk, identity=ident[:])
                    h1T = wp.tile([P, P], F32, tag="h1T")
                    nc.vector.tensor_copy(out=h1T[:], in_=tps[:])
                    hw_ps = pph.tile([P, H * D_H], F32, tag="hw")
                    nc.tensor.matmul(out=hw_ps[:], lhsT=h1T[:], rhs=w_gat[:],
                                     start=True, stop=True)
                    tw = wp.tile([P, TBLW], F32, tag="tw")
                    nc.vector.tensor_copy(out=tw[:, 0:H * D_H], in_=hw_ps[:])
                    scr = wp.tile([P, H * D_H], F32, tag="scr")
                    nc.vector.tensor_tensor(out=scr[:], in0=tw[:, 0:H * D_H],
                                            in1=att_s[:], op=ALU.mult)
                    nc.vector.reduce_sum(
                        out=tw[:, 256:258].rearrange("p (a b) -> p a b", b=1),
                        in_=scr[:].rearrange("p (a c) -> p a c", c=D_H),
                        axis=mybir.AxisListType.X)
                    nc.vector.tensor_tensor(out=scr[:], in0=tw[:, 0:H * D_H],
                                            in1=att_dt[:], op=ALU.mult)
                    with nc.allow_low_precision(reason="a_d stored bf16"):
                        nc.vector.reduce_sum(
                            out=ad_sb[:, 2 * b:2 * b + 2].rearrange(
                                "p (a b) -> p a b", b=1),
                            in_=scr[:].rearrange("p (a c) -> p a c", c=D_H),
                            axis=mybir.AxisListType.X)
                    nc.scalar.dma_start(out=hwt_slab[b * P:(b + 1) * P, :], in_=tw[:])

            nc.gpsimd.collective_compute(
                "AllGather", ALU.bypass, replica_groups=rg,
                ins=[hwt_slab.opt()], outs=[hwt_full.opt()])

            # =============== Layer 2: GAT ===============
            with (
                tc.tile_pool(name="l2w", bufs=2) as wp,
                tc.tile_pool(name="l2g", bufs=2) as gp,
                tc.tile_pool(name="l2t", bufs=2) as tp2,
                tc.tile_pool(name="l2p", bufs=2, space="PSUM") as pp,
                tc.tile_pool(name="l2pc", bufs=2, space="PSUM") as ppc,
                tc.tile_pool(name="l2pa", bufs=2, space="PSUM") as ppa,
            ):
                NCHUNK = (T * P + 511) // 512
                for b in range(BPC):
                    g0 = gp.tile([P, Th * TBLW], F32, tag="g2a")
                    g1 = gp.tile([P, Th * TBLW], F32, tag="g2b")
                    for h, g in ((0, g0), (1, g1)):
                        src_ap = hwt_full[:] if h == 0 else hwt_full[HALF:, :]
                        ixt = gp.tile([P, NI // 16], I16, tag="ix2")
                        nc.sync.dma_start(out=ixt[:], in_=idx_d[b, h])
                        nc.gpsimd.dma_gather(
                            out_ap=g[:].rearrange("p (t w) -> p t w", w=TBLW),
                            in_ap=src_ap,
                            idxs_ap=ixt[:],
                            num_idxs=NI, num_idxs_reg=NI, elem_size=TBLW,
                            single_packet=False, queue_num=next_q())
                    mT = wp.tile([1, T * P], F32, tag="mT")
                    nc.sync.dma_start(out=mT[:], in_=metaT_d[b][None, :])
                    ad_ps = ppa.tile([P, 2 * T], F32, tag="adp")
                    for c in range(NCHUNK):
                        c0 = c * 512
                        c1 = min(T * P, c0 + 512)
                        cps = ppc.tile([P, 512], F32, tag="ck")
                        nc.tensor.matmul(out=cps[:, 0:c1 - c0], lhsT=onesr[:],
                                         rhs=mT[:, c0:c1], start=True, stop=True)
                        ohc = tp2.tile([P, 512], BF16, tag="ohT")
                        nc.vector.tensor_scalar(
                            out=ohc[:, 0:c1 - c0], in0=cps[:, 0:c1 - c0],
                            scalar1=iotac[:], scalar2=None, op0=ALU.is_equal)
                        for t in range(c0 // P, c1 // P):
                            nc.tensor.matmul(
                                out=ad_ps[:, 2 * t:2 * t + 2],
                                lhsT=ohc[:, t * P - c0:(t + 1) * P - c0],
                                rhs=ad_sb[:, 2 * b:2 * b + 2],
                                start=True, stop=True)
                    sc = wp.tile([P, 2 * T], F32, tag="sc")
                    for h, g in ((0, g0), (1, g1)):
                        nc.vector.tensor_tensor(
                            out=sc[:, h * 2 * Th:(h + 1) * 2 * Th].rearrange(
                                "p (t two) -> p t two", two=2),
                            in0=g[:].rearrange("p (t w) -> p t w", w=TBLW)[:, :, 256:258],
                            in1=ad_ps[:, h * 2 * Th:(h + 1) * 2 * Th].rearrange(
                                "p (t two) -> p t two", two=2),
                            op=ALU.add)
                    sc2 = wp.tile([P, 2 * T], F32, tag="sc2")
                    nc.vector.tensor_scalar(out=sc2[:], in0=sc[:], scalar1=NEG_SLOPE,
                                            scalar2=None, op0=ALU.mult)
                    nc.vector.tensor_tensor(out=sc[:], in0=sc[:], in1=sc2[:],
                                            op=ALU.max)
                    ex = wp.tile([P, 2 * T], F32, tag="ex")
                    nc.scalar.activation(out=ex[:], in_=sc[:], func=ACTF.Exp)
                    gat_ps = pp.tile([P, H * D_H + 2], F32, tag="pgat")
                    for t in range(T):
                        h, tr = divmod(t, Th)
                        g = g0 if h == 0 else g1
                        oh = wp.tile([P, P], F32, tag="oh2")
                        nc.vector.tensor_scalar(
                            out=oh[:], in0=iotar[:], scalar1=mcol(b, t),
                            scalar2=None, op0=ALU.is_equal)
                        mw = wp.tile([P, H * D_H + 2], F32, tag="mw")
                        nc.vector.tensor_scalar(
                            out=mw[:, 0:D_H], in0=g[:, tr * TBLW:tr * TBLW + D_H],
                            scalar1=ex[:, 2 * t:2 * t + 1], scalar2=None, op0=ALU.mult)
                        nc.vector.tensor_scalar(
                            out=mw[:, D_H:2 * D_H],
                            in0=g[:, tr * TBLW + D_H:tr * TBLW + 2 * D_H],
                            scalar1=ex[:, 2 * t + 1:2 * t + 2], scalar2=None,
                            op0=ALU.mult)
                        nc.vector.tensor_copy(out=mw[:, 2 * D_H:2 * D_H + 2],
                                              in_=ex[:, 2 * t:2 * t + 2])
                        nc.tensor.matmul(out=gat_ps[:], lhsT=oh[:], rhs=mw[:],
                                         start=(t == 0), stop=(t == T - 1))
                    s2 = wp.tile([P, 2], F32, tag="s2")
                    nc.vector.tensor_scalar(out=s2[:], in0=gat_ps[:, 256:258],
                                            scalar1=1e-30, scalar2=None, op0=ALU.add)
                    rec = wp.tile([P, 2], F32, tag="rec")
                    nc.vector.reciprocal(out=rec[:], in_=s2[:])
                    t0 = wp.tile([P, D_H], F32, tag="t0")
                    nc.vector.tensor_scalar(out=t0[:], in0=gat_ps[:, 0:D_H],
                                            scalar1=rec[:, 0:1], scalar2=None,
                                            op0=ALU.mult)
                    t1 = wp.tile([P, D_H], F32, tag="t1")
                    nc.vector.tensor_scalar(out=t1[:], in0=gat_ps[:, D_H:2 * D_H],
                                            scalar1=rec[:, 1:2], scalar2=None,
                                            op0=ALU.mult)
                    u2 = wp.tile([P, D_H], F32, tag="u2")
                    nc.vector.tensor_tensor(out=u2[:], in0=t0[:], in1=t1[:], op=ALU.add)
                    h2_blk = h2_sb[:, b * P:(b + 1) * P]
                    nc.scalar.activation(out=h2_blk, in_=u2[:], func=ACTF.Relu,
                                         scale=0.5)
                    nc.scalar.dma_start(out=h2_slab[b * P:(b + 1) * P, :], in_=h2_blk)

            nc.gpsimd.collective_compute(
                "AllGather", ALU.bypass, replica_groups=rg,
                ins=[h2_slab.opt()], outs=[h2_full.opt()])

            # =============== Layer 3: SAGE + output ===============
            with (
                tc.tile_pool(name="l3w", bufs=3) as wp,
                tc.tile_pool(name="l3p", bufs=2, space="PSUM") as pp,
                tc.tile_pool(name="l3pt", bufs=2, space="PSUM") as ppt,
                tc.tile_pool(name="l3po", bufs=1, space="PSUM") as ppo,
                tc.tile_pool(name="l3pl", bufs=1, space="PSUM") as ppl,
            ):
                for b in range(BPC):
                    g0 = wp.tile([P, Th * D_H], F32, tag="g3a")
                    g1 = wp.tile([P, Th * D_H], F32, tag="g3b")
                    for h, g in ((0, g0), (1, g1)):
                        src_ap = h2_full[:] if h == 0 else h2_full[HALF:, :]
                        ixt = wp.tile([P, NI // 16], I16, tag="ix3")
                        nc.sync.dma_start(out=ixt[:], in_=idx_d[b, h])
                        nc.gpsimd.dma_gather(
                            out_ap=g[:].rearrange("p (t w) -> p t w", w=D_H),
                            in_ap=src_ap,
                            idxs_ap=ixt[:],
                            num_idxs=NI, num_idxs_reg=NI, elem_size=D_H,
                            single_packet=False, queue_num=next_q())
                    psum = pp.tile([P, D_H], F32, tag="ps")
                    for t in range(T):
                        h, tr = divmod(t, Th)
                        g = g0 if h == 0 else g1
                        oh = wp.tile([P, P], F32, tag="oh3")
                        nc.vector.tensor_scalar(
                            out=oh[:], in0=iotar[:], scalar1=msage(b, t),
                            scalar2=None, op0=ALU.is_equal)
                        nc.tensor.matmul(out=psum[:], lhsT=oh[:],
                                         rhs=g[:, tr * D_H:(tr + 1) * D_H],
                                         start=(t == 0), stop=(t == T - 1))
                    recd = wp.tile([P, 1], F32, tag="recd")
                    nc.vector.reciprocal(out=recd[:],
                                         in_=degs_res[:, 2 * b + 1:2 * b + 2])
                    agg = wp.tile([P, D_H], F32, tag="agg")
                    nc.vector.tensor_scalar(out=agg[:], in0=psum[:], scalar1=recd[:],
                                            scalar2=None, op0=ALU.mult)
                    tps = ppt.tile([P, P], F32, tag="tr3")
                    nc.tensor.transpose(out=tps[:], in_=agg[:], identity=ident[:])
                    aggT = wp.tile([P, P], F32, tag="aggT")
                    nc.vector.tensor_copy(out=aggT[:], in_=tps[:])
                    tps2 = ppt.tile([P, P], F32, tag="tr3")
                    nc.tensor.transpose(out=tps2[:], in_=h2_sb[:, b * P:(b + 1) * P],
                                        identity=ident[:])
                    h2T = wp.tile([P, P], F32, tag="h2T")
                    nc.vector.tensor_copy(out=h2T[:], in_=tps2[:])
                    ops = ppo.tile([P, D_H], F32, tag="po")
                    nc.tensor.matmul(out=ops[:], lhsT=aggT[:], rhs=w_sl[:],
                                     start=True, stop=False)
                    nc.tensor.matmul(out=ops[:], lhsT=h2T[:], rhs=w_sr[:],
                                     start=False, stop=True)
                    h3 = h1_sb[:, b * P:(b + 1) * P]  # reuse h1 slab for h3
                    nc.scalar.activation(out=h3, in_=ops[:], func=ACTF.Relu)
                    tps3 = ppt.tile([P, P], F32, tag="tr3")
                    nc.tensor.transpose(out=tps3[:], in_=h3, identity=ident[:])
                    h3T = wp.tile([P, P], F32, tag="h3T")
                    nc.vector.tensor_copy(out=h3T[:], in_=tps3[:])
                    lg = ppl.tile([P, D_OUT], F32, tag="lg")
                    nc.tensor.matmul(out=lg[:], lhsT=h3T[:], rhs=w_out[:],
                                     start=True, stop=True)
                    m = wp.tile([P, 1], F32, tag="m")
                    nc.vector.reduce_max(out=m[:], in_=lg[:], axis=mybir.AxisListType.X)
                    tl = wp.tile([P, D_OUT], F32, tag="tl")
                    nc.vector.tensor_scalar(out=tl[:], in0=lg[:], scalar1=m[:],
                                            scalar2=None, op0=ALU.subtract)
                    epx = wp.tile([P, D_OUT], F32, tag="epx")
                    nc.scalar.activation(out=epx[:], in_=tl[:], func=ACTF.Exp)
                    sacc = wp.tile([P, 1], F32, tag="sacc")
                    nc.vector.reduce_sum(out=sacc[:], in_=epx[:],
                                         axis=mybir.AxisListType.X)
                    lse = wp.tile([P, 1], F32, tag="lse")
                    nc.scalar.activation(out=lse[:], in_=sacc[:], func=ACTF.Ln)
                    ob = wp.tile([P, D_OUT], F32, tag="ob")
                    nc.vector.tensor_scalar(out=ob[:], in0=tl[:], scalar1=lse[:],
                                            scalar2=None, op0=ALU.subtract)
                    nc.sync.dma_start(out=out_d[b * P:(b + 1) * P, :], in_=ob[:])

    nc.compile()
    return nc


# ----------------------------------------------------------------------------
# Entry point
# ----------------------------------------------------------------------------

def kernel(x, W_gcn, b_gcn, W_gat, att_src, att_dst, b_gat,
           W_sage_l, b_sage_l, W_sage_r, W_out, b_out, edge_index):
    x = np.asarray(x, np.float32)
    N = x.shape[0]
    for bb in (b_gcn, b_gat, b_sage_l, b_out):
        assert not np.any(np.asarray(bb)), "nonzero biases not wired in"
    pk = _pack(np.asarray(edge_index), N)
    NPAD, BPC = pk["NPAD"], pk["BPC"]

    x_perm = np.zeros((NPAD, D_IN), np.float32)
    x_perm[pk["perm"]] = x

    nc = _build_program(pk)

    att_s_b = np.tile(np.asarray(att_src, np.float32).reshape(1, H * D_H),
                      (P, 1)).copy()
    att_d_b = np.tile(np.asarray(att_dst, np.float32).reshape(1, H * D_H),
                      (P, 1)).copy()
    common = {
        "x_perm": x_perm,
        "w_gcn": np.ascontiguousarray(W_gcn, np.float32),
        "w_gat": np.ascontiguousarray(W_gat, np.float32),
        "att_s": att_s_b, "att_d": att_d_b,
        "w_sl": np.ascontiguousarray(W_sage_l, np.float32),
        "w_sr": np.ascontiguousarray(W_sage_r, np.float32),
        "w_out": np.ascontiguousarray(W_out, np.float32),
        "ident": np.eye(P, dtype=np.float32),
        "iotar": np.ascontiguousarray(
            np.tile(np.arange(P, dtype=np.float32)[None, :], (P, 1))),
        "iotac": np.ascontiguousarray(np.arange(P, dtype=np.float32)[:, None]),
        "onesr": np.ones((1, P), np.float32),
    }
    in_maps = []
    for c in range(NC):
        m = dict(common)
        m["idx"] = np.ascontiguousarray(pk["idx"][c * BPC:(c + 1) * BPC])
        m["meta"] = np.ascontiguousarray(pk["meta"][c * BPC:(c + 1) * BPC])
        m["metaT"] = np.ascontiguousarray(pk["metaT"][c * BPC:(c + 1) * BPC])
        m["degs"] = np.ascontiguousarray(pk["degs"][c * BPC:(c + 1) * BPC])
        in_maps.append(m)

    trace = bool(os.environ.get("GNN_KERNEL_TRACE"))
    if trace:
        _install_ntff_shim()
    res = run_bass_kernel_spmd(nc, in_maps, core_ids=list(range(NC)), trace=trace)
    if trace and res.exec_time_ns:
        print(f"HW exec time: {res.exec_time_ns} ns")

    out_all = np.concatenate([r["out"] for r in res.results], axis=0)
    return np.ascontiguousarray(out_all[pk["perm"]].astype(np.float32))


def _install_ntff_shim():
    import types
    try:
        from antenv import axon_hooks  # noqa: F401
        return
    except ImportError:
        pass
    import antenv
    mod = types.ModuleType("antenv.axon_hooks")
    mod._hook = None
    mod.set_axon_ntff_profile_hook = lambda h: setattr(mod, "_hook", h)
    mod.get_axon_ntff_profile_hook = lambda: mod._hook
    sys.modules["antenv.axon_hooks"] = mod
    antenv.axon_hooks = mod
    try:
        from trn_agent_boot.trn_boot import _ntff_profile_via_ctypes
        hook = _ntff_profile_via_ctypes("/opt/axon/libaxon_pjrt.so")
        if hook is not None:
            mod.set_axon_ntff_profile_hook(hook)
    except Exception:
        pass



# revision 8
# speedup vs baseline: 1.8436x; 1.8436x over previous
"""Trainium2 Bass kernel for the MixedGNN problem (GCN -> GAT -> SAGE -> linear+log_softmax).

v2 design, driven by trace analysis of the v1 baseline (5.62 ms):
the dominant cost was SWDGE dma_gather descriptor generation on GpSimd
(~8.2 ns per gathered row, serialized) plus fp32 one-hot scatter matmuls
and DVE one-hot builds slowed 8x by concurrent gather SBUF writes.

Changes:
- Layer 1 (GCN) gathers are eliminated: the host pre-stages x[src] in
  edge-slot order (bf16), loaded with dense HWDGE descriptors.
- All feature tables, one-hot matrices, and matmuls are bf16 (PE runs
  4x faster than fp32; DVE 2x).
- GAT aggregates in h1-space (128 wide) using linearity of the head
  projection; gathered table rows are [h1 | 1.0 | a_s0 a_s1 | pad] at
  512 B so the softmax numerator and denominator come from one matmul.
- Self-loop edges of GCN come from the host stage; GAT self-attention is
  applied locally per block (no gathered self rows); SAGE has no self
  loops. L2 and L3 share one self-loop-free edge slotting and idx tiles.
- AllGathers ship bf16 tables chunk-major (7 chunks) so they overlap the
  producing layer's block loop.

Host-side work is layout only (permutation / duplication / dtype cast);
all floating-point model math runs on the NeuronCores.
"""

import os
import sys
import heapq

import numpy as np

sys.path.insert(0, "/opt/trn_rl_repo")

import ml_dtypes  # noqa: E402

import concourse.tile as tile  # noqa: E402
from concourse import bacc, mybir  # noqa: E402
from concourse.bass_utils import run_bass_kernel_spmd  # noqa: E402

F32 = mybir.dt.float32
BF16 = mybir.dt.bfloat16
I16 = mybir.dt.int16
ALU = mybir.AluOpType
ACTF = mybir.ActivationFunctionType
BF = ml_dtypes.bfloat16

NC = 8
P = 128
D = 128          # D_IN == D_H
H = 2
D_OUT = 32
NEG_SLOPE = 0.2
TBLW = 256       # GAT table row: [h1(128) | 1.0 | a_s0 a_s1 | zeros] bf16 = 512B
NCH = 7          # AllGather chunks (7 blocks each per core)


# ----------------------------------------------------------------------------
# Host packing (layout only)
# ----------------------------------------------------------------------------

def _assign_blocks(w, nblk, rng):
    """Greedy balanced assignment of nodes to blocks (<=128 nodes each)."""
    n = len(w)
    order = np.lexsort((rng.permutation(n), -w))
    blk_of = np.empty(n, np.int32)
    heap = [(0, b) for b in range(nblk)]
    heapq.heapify(heap)
    nodecnt = np.zeros(nblk, np.int32)
    for i in order:
        load, b = heapq.heappop(heap)
        blk_of[i] = b
        nodecnt[b] += 1
        if nodecnt[b] < P:
            heapq.heappush(heap, (load + int(w[i]), b))
    return blk_of


def _chunked_addr(pblk, slot, BPC):
    """Map (global block, slot) -> chunk-major DRAM row address."""
    core = pblk // BPC
    j = pblk % BPC
    ch = j // (BPC // NCH)
    jj = j % (BPC // NCH)
    return ((ch * NC + core) * (BPC // NCH) + jj) * P + slot


def _pack(edge_index, N):
    E = edge_index.shape[1]
    src = np.asarray(edge_index[0], dtype=np.int64)
    dst = np.asarray(edge_index[1], dtype=np.int64)
    NBLK = NC * NCH * int(np.ceil(N / (P * NC * NCH)))
    NPAD = NBLK * P
    HALF = NPAD // 2
    BPC = NBLK // NC
    SLAB = BPC * P

    deg_in = np.bincount(dst, minlength=N).astype(np.int64)

    best = None
    rng = np.random.default_rng(1234)
    for _try in range(8):
        blk_of = _assign_blocks(deg_in + 1, NBLK, rng)
        order = np.argsort(blk_of, kind="stable")
        cnt = np.bincount(blk_of, minlength=NBLK)
        starts = np.zeros(NBLK + 1, np.int64)
        np.cumsum(cnt, out=starts[1:])
        slot = np.arange(N) - starts[blk_of[order]]
        perm = np.empty(N, np.int64)
        perm[order] = blk_of[order] * P + slot
        pblk = perm // P
        # chunk-major address of every node (for gather tables)
        caddr = _chunked_addr(pblk, perm % P, BPC)
        psrc_c = caddr[src]
        pdst = perm[dst]
        # L2/L3 grouping: (dst block, src half by chunked addr)
        key = (pdst >> 7) * 2 + (psrc_c >= HALF)
        counts = np.bincount(key, minlength=NBLK * 2)
        t2 = int(np.ceil(counts.max() / P))
        # L1 grouping: dst block, incl self loops
        cnt1 = np.bincount(perm[dst] >> 7, minlength=NBLK) + cnt
        t1 = int(np.ceil(cnt1.max() / P))
        score = 2 * t2 + t1
        if best is None or score < best[0]:
            best = (score, t1, t2, perm, caddr, counts)
        if t2 <= int(np.ceil(counts.mean() / P)) and \
           t1 <= int(np.ceil(cnt1.mean() / P)):
            break
    _, T1, t2, perm, caddr, counts = best
    T2 = 2 * t2
    SLOT2 = t2 * P

    pblk = perm // P
    pdst = perm[dst]
    psrc_c = caddr[src]

    # ---------------- L2/L3 slotting (no self loops) ----------------
    key = (pdst >> 7) * 2 + (psrc_c >= HALF)
    ordr = np.lexsort((psrc_c, key))
    ks = key[ordr]
    grp_start = np.concatenate(([0], np.cumsum(counts)))[ks]
    pos_in_grp = np.arange(len(ks)) - grp_start
    slot_pos = ks * SLOT2 + pos_in_grp

    tot = NBLK * 2 * SLOT2
    eidx = np.zeros(tot, np.int64)             # pad rows gather row 0 (excluded by edl=-1)
    edl = np.full(tot, -1.0, np.float32)
    eidx[slot_pos] = psrc_c[ordr] - (ks % 2) * HALF
    edl[slot_pos] = (pdst[ordr] & 127).astype(np.float32)
    assert eidx.max() < HALF and eidx.min() >= 0
    eidx16 = eidx.astype(np.int16)

    # idx tiles: flat i -> [i%16, i//16], replicated x8 down partitions
    A = eidx16.reshape(NBLK, 2, SLOT2 // 16, 16).transpose(0, 1, 3, 2)
    idx2 = np.ascontiguousarray(np.tile(A, (1, 1, 8, 1)))

    # edl per block: [P, T2] bf16  (edge at (p, t) = slot t*128+p)
    edl2 = np.ascontiguousarray(
        edl.reshape(NBLK, T2, P).transpose(0, 2, 1).astype(np.float32))
    # flat dst-slot list per block for a_d broadcast: [T2*P] bf16
    metaT2 = np.ascontiguousarray(edl.reshape(NBLK, T2 * P).astype(BF))

    # ---------------- L1 slotting (with self loops), host pre-gather ----
    esrc1 = np.concatenate([src, np.arange(N)])
    edst1 = np.concatenate([dst, np.arange(N)])
    pdst1 = perm[edst1]
    key1 = pdst1 >> 7
    ordr1 = np.lexsort((esrc1, key1))
    ks1 = key1[ordr1]
    cnt1 = np.bincount(key1, minlength=NBLK)
    grp1 = np.concatenate(([0], np.cumsum(cnt1)))[ks1]
    pos1 = np.arange(len(ks1)) - grp1
    spos1 = ks1 * (T1 * P) + pos1

    tot1 = NBLK * T1 * P
    e1src = np.zeros(tot1, np.int64)           # gather source node (orig id)
    e1dl = np.full(tot1, -1.0, np.float32)
    e1w = np.ones(tot1, np.float32)
    e1src[spos1] = esrc1[ordr1]
    e1dl[spos1] = (pdst1[ordr1] & 127).astype(np.float32)
    e1w[spos1] = (deg_in + 1)[esrc1[ordr1]].astype(np.float32)

    edl1 = np.ascontiguousarray(
        e1dl.reshape(NBLK, T1, P).transpose(0, 2, 1).astype(np.float32))
    ew1 = np.ascontiguousarray(
        e1w.reshape(NBLK, T1, P).transpose(0, 2, 1).astype(np.float32))
    # xe layout [NBLK, P, T1, D]: edge j=t*128+p of block b -> [b, p, t, :]
    xe_map = np.ascontiguousarray(
        e1src.reshape(NBLK, T1, P).transpose(0, 2, 1))

    # per-node degrees [NBLK, P, 2]: (deg+1 for GCN, max(deg,1) for SAGE)
    w_p = np.ones(NPAD, np.float32)
    w_p[perm] = (deg_in + 1).astype(np.float32)
    sg_p = np.ones(NPAD, np.float32)
    sg_p[perm] = np.maximum(deg_in, 1).astype(np.float32)
    degs = np.ascontiguousarray(
        np.stack([w_p.reshape(NBLK, P), sg_p.reshape(NBLK, P)], axis=2))

    return dict(
        NBLK=NBLK, NPAD=NPAD, HALF=HALF, BPC=BPC, SLAB=SLAB,
        T1=T1, t2=t2, T2=T2, perm=perm,
        idx2=idx2, edl2=edl2, metaT2=metaT2,
        edl1=edl1, ew1=ew1, xe_map=xe_map, degs=degs,
    )


# ----------------------------------------------------------------------------
# Device program
# ----------------------------------------------------------------------------

def _build_program(pk):
    BPC, T1, t2, T2, NPAD, HALF, SLAB = (
        pk["BPC"], pk["T1"], pk["t2"], pk["T2"],
        pk["NPAD"], pk["HALF"], pk["SLAB"])
    NI2 = t2 * P
    BPCH = BPC // NCH           # blocks per AG chunk (7)
    CHROW = BPCH * P            # slab rows per chunk
    NCHUNK = (T2 * P + 511) // 512

    nc = bacc.Bacc("TRN2", target_bir_lowering=False, num_devices=NC,
                   num_swdge_queues=4, dynamic_dma_scratch_size=65536)

    xe_d = nc.dram_tensor("xe", [BPC, P, T1 * D], BF16, kind="ExternalInput")
    idx_d = nc.dram_tensor("idx", [BPC, 2, P, NI2 // 16], I16,
                           kind="ExternalInput")
    edl1_d = nc.dram_tensor("edl1", [BPC, P, T1], F32, kind="ExternalInput")
    ew1_d = nc.dram_tensor("ew1", [BPC, P, T1], F32, kind="ExternalInput")
    edl2_d = nc.dram_tensor("edl2", [BPC, P, T2], F32, kind="ExternalInput")
    metaT2_d = nc.dram_tensor("metaT2", [BPC, T2 * P], BF16,
                              kind="ExternalInput")
    degs_d = nc.dram_tensor("degs", [BPC, P, 2], F32, kind="ExternalInput")
    w_gcn_d = nc.dram_tensor("w_gcn", [D, D], BF16, kind="ExternalInput")
    w_gat_d = nc.dram_tensor("w_gat", [D, H * D], BF16, kind="ExternalInput")
    w_gat_f_d = nc.dram_tensor("w_gat_f", [D, H * D], F32, kind="ExternalInput")
    att_s_d = nc.dram_tensor("att_s", [P, H * D], F32, kind="ExternalInput")
    att_d_d = nc.dram_tensor("att_d", [P, H * D], F32, kind="ExternalInput")
    w_sl_d = nc.dram_tensor("w_sl", [D, D], BF16, kind="ExternalInput")
    w_sr_d = nc.dram_tensor("w_sr", [D, D], BF16, kind="ExternalInput")
    w_out_d = nc.dram_tensor("w_out", [D, D_OUT], BF16, kind="ExternalInput")
    ident_d = nc.dram_tensor("ident", [P, P], BF16, kind="ExternalInput")
    iotar_d = nc.dram_tensor("iotar", [P, P], BF16, kind="ExternalInput")
    iotac_d = nc.dram_tensor("iotac", [P, 1], F32, kind="ExternalInput")
    onesr_d = nc.dram_tensor("onesr", [1, P], BF16, kind="ExternalInput")
    onesc_d = nc.dram_tensor("onesc", [P, 1], BF16, kind="ExternalInput")
    out_d = nc.dram_tensor("out", [SLAB, D_OUT], F32, kind="ExternalOutput")

    rg = [list(range(NC))]
    qn = [0]

    def next_q():
        qn[0] = (qn[0] + 1) % 4
        return qn[0]

    with tile.TileContext(nc) as tc:
        with (
            tc.tile_pool(name="const", bufs=1) as cp,
            tc.tile_pool(name="dram", bufs=1, space="DRAM") as dp,
        ):
            def cload(shape, dt, src, tag):
                t = cp.tile(shape, dt, tag=tag)
                nc.sync.dma_start(out=t[:], in_=src)
                return t

            w_gcn = cload([D, D], BF16, w_gcn_d[:], "c_wgcn")
            w_gat = cload([D, H * D], BF16, w_gat_d[:], "c_wgat")
            w_gat_f = cload([D, H * D], F32, w_gat_f_d[:], "c_wgatf")
            att_s = cload([P, H * D], F32, att_s_d[:], "c_atts")
            att_dt = cload([P, H * D], F32, att_d_d[:], "c_attd")
            w_sl = cload([D, D], BF16, w_sl_d[:], "c_wsl")
            w_sr = cload([D, D], BF16, w_sr_d[:], "c_wsr")
            w_out = cload([D, D_OUT], BF16, w_out_d[:], "c_wout")
            ident = cload([P, P], BF16, ident_d[:], "c_ident")
            iotar = cload([P, P], BF16, iotar_d[:], "c_iotar")
            iotac = cload([P, 1], F32, iotac_d[:], "c_iotac")
            onesr = cload([1, P], BF16, onesr_d[:], "c_onesr")
            onesc = cload([P, 1], BF16, onesc_d[:], "c_onesc")

            edl1_res = cp.tile([P, BPC * T1], F32)
            edl2_res = cp.tile([P, BPC * T2], F32)
            for b in range(BPC):
                nc.sync.dma_start(out=edl1_res[:, b * T1:(b + 1) * T1],
                                  in_=edl1_d[b])
                nc.sync.dma_start(out=edl2_res[:, b * T2:(b + 1) * T2],
                                  in_=edl2_d[b])
            degs_res = cp.tile([P, BPC * 2], F32)
            for b in range(BPC):
                nc.sync.dma_start(out=degs_res[:, b * 2:(b + 1) * 2],
                                  in_=degs_d[b])

            h1_sb = cp.tile([P, BPC * P], BF16)   # h1 slab (reused for h3)
            h2_sb = cp.tile([P, BPC * P], BF16)   # h2 slab
            ad_sb = cp.tile([P, 2 * BPC], BF16)   # per-own-node a_d
            as_sb = cp.tile([P, 2 * BPC], BF16)   # per-own-node a_s

            hwt_slab = dp.tile([SLAB, TBLW], BF16)
            hwt_full = dp.tile([NPAD, TBLW], BF16)
            h2_slab = dp.tile([SLAB, D], BF16)
            h2_full = dp.tile([NPAD, D], BF16)

            def m1col(b, t):
                return edl1_res[:, b * T1 + t:b * T1 + t + 1]

            def m2col(b, t):
                return edl2_res[:, b * T2 + t:b * T2 + t + 1]

            # v = [v_s0 v_s1 v_d0 v_d1]: v_s[c,h] = sum_c' W_gat[c, h*D+c']*att_s[h,c']
            vprep = cp.tile([P, H * D], F32)
            v_sd = cp.tile([P, 4], BF16)
            nc.vector.tensor_tensor(out=vprep[:], in0=w_gat_f[:], in1=att_s[:],
                                    op=ALU.mult)
            with nc.allow_low_precision(reason="a_s proj bf16"):
                nc.vector.reduce_sum(
                    out=v_sd[:, 0:2].rearrange("p (a b) -> p a b", b=1),
                    in_=vprep[:].rearrange("p (a c) -> p a c", c=D),
                    axis=mybir.AxisListType.X)
            nc.vector.tensor_tensor(out=vprep[:], in0=w_gat_f[:], in1=att_dt[:],
                                    op=ALU.mult)
            with nc.allow_low_precision(reason="a_d proj bf16"):
                nc.vector.reduce_sum(
                    out=v_sd[:, 2:4].rearrange("p (a b) -> p a b", b=1),
                    in_=vprep[:].rearrange("p (a c) -> p a c", c=D),
                    axis=mybir.AxisListType.X)

            # =============== Layer 1: GCN + table build ===============
            with (
                tc.tile_pool(name="l1x", bufs=2) as xp,
                tc.tile_pool(name="l1w", bufs=2) as wp,
                tc.tile_pool(name="l1tw", bufs=2) as twp,
                tc.tile_pool(name="l1p", bufs=2, space="PSUM") as pp,
                tc.tile_pool(name="l1pt", bufs=2, space="PSUM") as ppt,
                tc.tile_pool(name="l1ph", bufs=2, space="PSUM") as pph,
            ):
                # zero the tw pool buffers once (cols >131 stay zero)
                tw_bufs = []
                for i in range(2):
                    tw = twp.tile([P, TBLW], BF16, tag="tw")
                    nc.vector.memset(tw[:], 0.0)
                    tw_bufs.append(tw)

                for b in range(BPC):
                    xe = xp.tile([P, T1 * D], BF16, tag="xe")
                    nc.sync.dma_start(out=xe[:], in_=xe_d[b])
                    ew = wp.tile([P, T1], F32, tag="ew")
                    nc.sync.dma_start(out=ew[:], in_=ew1_d[b])
                    dinv = wp.tile([P, T1], F32, tag="dinv")
                    nc.scalar.activation(out=dinv[:], in_=ew[:], func=ACTF.Sqrt)
                    nc.vector.reciprocal(out=dinv[:], in_=dinv[:])
                    psum = pp.tile([P, D], F32, tag="pg")
                    for t in range(T1):
                        oh = wp.tile([P, P], BF16, tag="oh1")
                        nc.vector.tensor_scalar(
                            out=oh[:], in0=iotar[:], scalar1=m1col(b, t),
                            scalar2=dinv[:, t:t + 1], op0=ALU.is_equal,
                            op1=ALU.mult)
                        nc.tensor.matmul(
                            out=psum[:], lhsT=oh[:],
                            rhs=xe[:, t * D:(t + 1) * D],
                            start=(t == 0), stop=(t == T1 - 1))
                    # epilogue: h1 = relu((dinv_o * psum) @ W_gcn)
                    dinv_o = wp.tile([P, 1], F32, tag="dv0")
                    nc.scalar.activation(out=dinv_o[:],
                                         in_=degs_res[:, 2 * b:2 * b + 1],
                                         func=ACTF.Sqrt)
                    nc.vector.reciprocal(out=dinv_o[:], in_=dinv_o[:])
                    pre = wp.tile([P, D], BF16, tag="pre")
                    nc.vector.tensor_scalar(out=pre[:], in0=psum[:],
                                            scalar1=dinv_o[:], scalar2=None,
                                            op0=ALU.mult)
                    tps0 = ppt.tile([P, P], BF16, tag="tr1")
                    nc.tensor.transpose(out=tps0[:], in_=pre[:], identity=ident[:])
                    preT = wp.tile([P, P], BF16, tag="preT")
                    nc.vector.tensor_copy(out=preT[:], in_=tps0[:])
                    gcn_ps = pph.tile([P, D], F32, tag="gc")
                    nc.tensor.matmul(out=gcn_ps[:], lhsT=preT[:], rhs=w_gcn[:],
                                     start=True, stop=True)
                    tw = tw_bufs[b % 2]
                    nc.scalar.activation(out=tw[:, 0:D], in_=gcn_ps[:],
                                         func=ACTF.Relu)
                    nc.vector.tensor_copy(out=tw[:, D:D + 1], in_=onesc[:])
                    h1_blk = h1_sb[:, b * P:(b + 1) * P]
                    nc.vector.tensor_copy(out=h1_blk, in_=tw[:, 0:D])
                    # a_s / a_d for own nodes
                    tps1 = ppt.tile([P, P], BF16, tag="tr1")
                    nc.tensor.transpose(out=tps1[:], in_=tw[:, 0:D],
                                        identity=ident[:])
                    h1T = wp.tile([P, P], BF16, tag="h1T")
                    nc.vector.tensor_copy(out=h1T[:], in_=tps1[:])
                    ab_ps = pph.tile([P, 4], F32, tag="ab")
                    nc.tensor.matmul(out=ab_ps[:], lhsT=h1T[:], rhs=v_sd[:],
                                     start=True, stop=True)
                    with nc.allow_low_precision(reason="a_sd bf16"):
                        nc.vector.tensor_copy(out=tw[:, D + 1:D + 3],
                                              in_=ab_ps[:, 0:2])
                        nc.vector.tensor_copy(out=as_sb[:, 2 * b:2 * b + 2],
                                              in_=ab_ps[:, 0:2])
                        nc.vector.tensor_copy(out=ad_sb[:, 2 * b:2 * b + 2],
                                              in_=ab_ps[:, 2:4])
                    nc.scalar.dma_start(out=hwt_slab[b * P:(b + 1) * P, :],
                                        in_=tw[:])
                    if b % BPCH == BPCH - 1:
                        ch = b // BPCH
                        nc.gpsimd.collective_compute(
                            "AllGather", ALU.bypass, replica_groups=rg,
                            ins=[hwt_slab[ch * CHROW:(ch + 1) * CHROW, :]],
                            outs=[hwt_full[ch * NC * CHROW:(ch + 1) * NC * CHROW, :]])

            # =============== Layer 2: GAT ===============
            with (
                tc.tile_pool(name="l2g", bufs=3) as gp,
                tc.tile_pool(name="l2w", bufs=2) as wp,
                tc.tile_pool(name="l2t", bufs=2) as tp2,
                tc.tile_pool(name="l2p", bufs=2, space="PSUM") as pp,
                tc.tile_pool(name="l2pc", bufs=2, space="PSUM") as ppc,
                tc.tile_pool(name="l2pt", bufs=1, space="PSUM") as ppt2,
                tc.tile_pool(name="l2pa", bufs=2, space="PSUM") as ppa,
                tc.tile_pool(name="l2pu", bufs=1, space="PSUM") as ppu,
            ):
                # zero gather buffers once (padding rows must stay finite)
                g_bufs = []
                for i in range(3):
                    g0 = gp.tile([P, t2 * TBLW], BF16, tag="g2a")
                    g1 = gp.tile([P, t2 * TBLW], BF16, tag="g2b")
                    nc.vector.memset(g0[:], 0.0)
                    nc.vector.memset(g1[:], 0.0)
                    g_bufs.append((g0, g1))

                for b in range(BPC):
                    g0, g1 = g_bufs[b % 3]
                    for h, g in ((0, g0), (1, g1)):
                        src_ap = hwt_full[:] if h == 0 else hwt_full[HALF:, :]
                        ixt = wp.tile([P, NI2 // 16], I16, tag="ix2")
                        nc.sync.dma_start(out=ixt[:], in_=idx_d[b, h])
                        nc.gpsimd.dma_gather(
                            out_ap=g[:].rearrange("p (t w) -> p t w", w=TBLW),
                            in_ap=src_ap,
                            idxs_ap=ixt[:],
                            num_idxs=NI2, num_idxs_reg=NI2, elem_size=TBLW,
                            single_packet=False, queue_num=next_q())
                    # per-edge a_d via one-hot-transpose chunks
                    mT = wp.tile([1, T2 * P], BF16, tag="mT")
                    nc.sync.dma_start(out=mT[:], in_=metaT2_d[b][None, :])
                    ad_ps = ppa.tile([P, 2 * T2], F32, tag="adp")
                    for c in range(NCHUNK):
                        c0 = c * 512
                        c1 = min(T2 * P, c0 + 512)
                        cps = ppc.tile([P, 512], F32, tag="ck")
                        nc.tensor.matmul(out=cps[:, 0:c1 - c0], lhsT=onesr[:],
                                         rhs=mT[:, c0:c1], start=True, stop=True)
                        ohc = tp2.tile([P, 512], BF16, tag="ohT")
                        nc.vector.tensor_scalar(
                            out=ohc[:, 0:c1 - c0], in0=cps[:, 0:c1 - c0],
                            scalar1=iotac[:], scalar2=None, op0=ALU.is_equal)
                        for t in range(c0 // P, c1 // P):
                            nc.tensor.matmul(
                                out=ad_ps[:, 2 * t:2 * t + 2],
                                lhsT=ohc[:, t * P - c0:(t + 1) * P - c0],
                                rhs=ad_sb[:, 2 * b:2 * b + 2],
                                start=True, stop=True)
                    # scores: e = leaky(a_s[src] + a_d[dst]); ex = exp(e)
                    sc = wp.tile([P, 2 * T2], BF16, tag="sc")
                    for h, g in ((0, g0), (1, g1)):
                        nc.vector.tensor_copy(
                            out=sc[:, h * t2 * 2:(h + 1) * t2 * 2].rearrange(
                                "p (t two) -> p t two", two=2),
                            in_=g[:].rearrange("p (t w) -> p t w", w=TBLW)[
                                :, :, D + 1:D + 3])
                    adb = wp.tile([P, 2 * T2], BF16, tag="adb")
                    nc.vector.tensor_copy(out=adb[:], in_=ad_ps[:])
                    # reorder ad (slot-major 2 per t) to match sc (half-major)
                    # sc layout: [h=0 slots t=0..t2-1, h=1 slots t=t2..T2-1]
                    # adb layout: per t pairs (h0,h1): [t, 2]
                    sc2 = wp.tile([P, 2 * T2], BF16, tag="sc2")
                    for h in range(2):
                        nc.vector.tensor_tensor(
                            out=sc2[:, h * t2 * 2:(h + 1) * t2 * 2].rearrange(
                                "p (t two) -> p t two", two=2),
                            in0=sc[:, h * t2 * 2:(h + 1) * t2 * 2].rearrange(
                                "p (t two) -> p t two", two=2),
                            in1=adb[:, 2 * h * t2:2 * (h + 1) * t2].rearrange(
                                "p (t two) -> p t two", two=2),
                            op=ALU.add)
                    lk = wp.tile([P, 2 * T2], BF16, tag="lk")
                    nc.vector.tensor_scalar(out=lk[:], in0=sc2[:],
                                            scalar1=NEG_SLOPE, scalar2=None,
                                            op0=ALU.mult)
                    nc.vector.tensor_tensor(out=lk[:], in0=sc2[:], in1=lk[:],
                                            op=ALU.max)
                    ex = wp.tile([P, 2 * T2], F32, tag="ex")
                    nc.scalar.activation(out=ex[:], in_=lk[:], func=ACTF.Exp)
                    # self-edge alpha
                    eself = wp.tile([P, 2], BF16, tag="esf")
                    nc.vector.tensor_tensor(out=eself[:],
                                            in0=as_sb[:, 2 * b:2 * b + 2],
                                            in1=ad_sb[:, 2 * b:2 * b + 2],
                                            op=ALU.add)
                    lsf = wp.tile([P, 2], BF16, tag="lsf")
                    nc.vector.tensor_scalar(out=lsf[:], in0=eself[:],
                                            scalar1=NEG_SLOPE, scalar2=None,
                                            op0=ALU.mult)
                    nc.vector.tensor_tensor(out=lsf[:], in0=eself[:], in1=lsf[:],
                                            op=ALU.max)
                    asf = wp.tile([P, 2], F32, tag="asf")
                    nc.scalar.activation(out=asf[:], in_=lsf[:], func=ACTF.Exp)
                    # self tile [h1 | 1]
                    st = tp2.tile([P, D + 1], BF16, tag="st")
                    nc.vector.tensor_copy(out=st[:, 0:D],
                                          in_=h1_sb[:, b * P:(b + 1) * P])
                    nc.vector.tensor_copy(out=st[:, D:D + 1], in_=onesc[:])
                    # weighted scatter per head
                    m_all = pp.tile([P, 2 * (D + 1)], F32, tag="m01")
                    m_ps = [m_all[:, 0:D + 1], m_all[:, D + 1:2 * (D + 1)]]
                    for t in range(T2):
                        h, tr = divmod(t, t2)
                        g = g0 if h == 0 else g1
                        for hh in range(2):
                            oha = wp.tile([P, P], BF16, tag="oh2")
                            # ex column for head hh at slot t
                            exc = ex[:, h * t2 * 2 + 2 * tr + hh:
                                     h * t2 * 2 + 2 * tr + hh + 1]
                            nc.vector.tensor_scalar(
                                out=oha[:], in0=iotar[:], scalar1=m2col(b, t),
                                scalar2=exc, op0=ALU.is_equal, op1=ALU.mult)
                            nc.tensor.matmul(
                                out=m_ps[hh], lhsT=oha[:],
                                rhs=g[:, tr * TBLW:tr * TBLW + D + 1],
                                start=(t == 0), stop=False)
                    for hh in range(2):
                        ohs = wp.tile([P, P], BF16, tag="ohs")
                        nc.vector.tensor_scalar(
                            out=ohs[:], in0=iotar[:], scalar1=iotac[:],
                            scalar2=asf[:, hh:hh + 1], op0=ALU.is_equal,
                            op1=ALU.mult)
                        nc.tensor.matmul(out=m_ps[hh], lhsT=ohs[:], rhs=st[:],
                                         start=False, stop=True)
                    # normalize, project per head, mean, relu
                    u_ps = ppu.tile([P, D], F32, tag="u")
                    for hh in range(2):
                        den = wp.tile([P, 1], F32, tag="den")
                        nc.vector.tensor_scalar(out=den[:],
                                                in0=m_ps[hh][:, D:D + 1],
                                                scalar1=1e-30, scalar2=None,
                                                op0=ALU.add)
                        rec = wp.tile([P, 1], F32, tag="rec")
                        nc.vector.reciprocal(out=rec[:], in_=den[:])
                        mn = wp.tile([P, D], BF16, tag="mn")
                        nc.vector.tensor_scalar(out=mn[:], in0=m_ps[hh][:, 0:D],
                                                scalar1=rec[:], scalar2=None,
                                                op0=ALU.mult)
                        tpsm = ppt2.tile([P, P], BF16, tag="trm")
                        nc.tensor.transpose(out=tpsm[:], in_=mn[:],
                                            identity=ident[:])
                        mnT = wp.tile([P, P], BF16, tag="mnT")
                        nc.vector.tensor_copy(out=mnT[:], in_=tpsm[:])
                        nc.tensor.matmul(out=u_ps[:], lhsT=mnT[:],
                                         rhs=w_gat[:, hh * D:(hh + 1) * D],
                                         start=(hh == 0), stop=(hh == 1))
                    h2_blk = h2_sb[:, b * P:(b + 1) * P]
                    nc.scalar.activation(out=h2_blk, in_=u_ps[:],
                                         func=ACTF.Relu, scale=0.5)
                    nc.scalar.dma_start(out=h2_slab[b * P:(b + 1) * P, :],
                                        in_=h2_blk)
                    if b % BPCH == BPCH - 1:
                        ch = b // BPCH
                        nc.gpsimd.collective_compute(
                            "AllGather", ALU.bypass, replica_groups=rg,
                            ins=[h2_slab[ch * CHROW:(ch + 1) * CHROW, :]],
                            outs=[h2_full[ch * NC * CHROW:(ch + 1) * NC * CHROW, :]])

            # =============== Layer 3: SAGE + output ===============
            with (
                tc.tile_pool(name="l3g", bufs=3) as gp,
                tc.tile_pool(name="l3w", bufs=2) as wp,
                tc.tile_pool(name="l3p", bufs=2, space="PSUM") as pp,
                tc.tile_pool(name="l3pt", bufs=2, space="PSUM") as ppt,
                tc.tile_pool(name="l3po", bufs=2, space="PSUM") as ppo,
            ):
                g_bufs = []
                for i in range(3):
                    g0 = gp.tile([P, t2 * D], BF16, tag="g3a")
                    g1 = gp.tile([P, t2 * D], BF16, tag="g3b")
                    nc.vector.memset(g0[:], 0.0)
                    nc.vector.memset(g1[:], 0.0)
                    g_bufs.append((g0, g1))

                for b in range(BPC):
                    g0, g1 = g_bufs[b % 3]
                    for h, g in ((0, g0), (1, g1)):
                        src_ap = h2_full[:] if h == 0 else h2_full[HALF:, :]
                        ixt = wp.tile([P, NI2 // 16], I16, tag="ix3")
                        nc.sync.dma_start(out=ixt[:], in_=idx_d[b, h])
                        nc.gpsimd.dma_gather(
                            out_ap=g[:].rearrange("p (t w) -> p t w", w=D),
                            in_ap=src_ap,
                            idxs_ap=ixt[:],
                            num_idxs=NI2, num_idxs_reg=NI2, elem_size=D,
                            single_packet=False, queue_num=next_q())
                    psum = pp.tile([P, D], F32, tag="ps")
                    for t in range(T2):
                        h, tr = divmod(t, t2)
                        g = g0 if h == 0 else g1
                        oh = wp.tile([P, P], BF16, tag="oh3")
                        nc.vector.tensor_scalar(
                            out=oh[:], in0=iotar[:], scalar1=m2col(b, t),
                            scalar2=None, op0=ALU.is_equal)
                        nc.tensor.matmul(out=psum[:], lhsT=oh[:],
                                         rhs=g[:, tr * D:(tr + 1) * D],
                                         start=(t == 0), stop=(t == T2 - 1))
                    recd = wp.tile([P, 1], F32, tag="recd")
                    nc.vector.reciprocal(out=recd[:],
                                         in_=degs_res[:, 2 * b + 1:2 * b + 2])
                    agg = wp.tile([P, D], BF16, tag="agg")
                    nc.vector.tensor_scalar(out=agg[:], in0=psum[:],
                                            scalar1=recd[:], scalar2=None,
                                            op0=ALU.mult)
                    tps = ppt.tile([P, P], BF16, tag="tr3")
                    nc.tensor.transpose(out=tps[:], in_=agg[:], identity=ident[:])
                    aggT = wp.tile([P, P], BF16, tag="aggT")
                    nc.vector.tensor_copy(out=aggT[:], in_=tps[:])
                    tps2 = ppt.tile([P, P], BF16, tag="tr3")
                    nc.tensor.transpose(out=tps2[:],
                                        in_=h2_sb[:, b * P:(b + 1) * P],
                                        identity=ident[:])
                    h2T = wp.tile([P, P], BF16, tag="h2T")
                    nc.vector.tensor_copy(out=h2T[:], in_=tps2[:])
                    ops = ppo.tile([P, D], F32, tag="po")
                    nc.tensor.matmul(out=ops[:], lhsT=aggT[:], rhs=w_sl[:],
                                     start=True, stop=False)
                    nc.tensor.matmul(out=ops[:], lhsT=h2T[:], rhs=w_sr[:],
                                     start=False, stop=True)
                    h3 = h1_sb[:, b * P:(b + 1) * P]  # reuse h1 slab
                    nc.scalar.activation(out=h3, in_=ops[:], func=ACTF.Relu)
                    tps3 = ppt.tile([P, P], BF16, tag="tr3")
                    nc.tensor.transpose(out=tps3[:], in_=h3, identity=ident[:])
                    h3T = wp.tile([P, P], BF16, tag="h3T")
                    nc.vector.tensor_copy(out=h3T[:], in_=tps3[:])
                    lg = ppo.tile([P, D_OUT], F32, tag="lg")
                    nc.tensor.matmul(out=lg[:], lhsT=h3T[:], rhs=w_out[:],
                                     start=True, stop=True)
                    m = wp.tile([P, 1], F32, tag="m")
                    nc.vector.reduce_max(out=m[:], in_=lg[:],
                                         axis=mybir.AxisListType.X)
                    tl = wp.tile([P, D_OUT], F32, tag="tl")
                    nc.vector.tensor_scalar(out=tl[:], in0=lg[:], scalar1=m[:],
                                            scalar2=None, op0=ALU.subtract)
                    epx = wp.tile([P, D_OUT], F32, tag="epx")
                    nc.scalar.activation(out=epx[:], in_=tl[:], func=ACTF.Exp)
                    sacc = wp.tile([P, 1], F32, tag="sacc")
                    nc.vector.reduce_sum(out=sacc[:], in_=epx[:],
                                         axis=mybir.AxisListType.X)
                    lse = wp.tile([P, 1], F32, tag="lse")
                    nc.scalar.activation(out=lse[:], in_=sacc[:], func=ACTF.Ln)
                    ob = wp.tile([P, D_OUT], F32, tag="ob")
                    nc.vector.tensor_scalar(out=ob[:], in0=tl[:], scalar1=lse[:],
                                            scalar2=None, op0=ALU.subtract)
                    nc.sync.dma_start(out=out_d[b * P:(b + 1) * P, :], in_=ob[:])

    nc.compile()
    return nc


# ----------------------------------------------------------------------------
# Entry point
# ----------------------------------------------------------------------------

def kernel(x, W_gcn, b_gcn, W_gat, att_src, att_dst, b_gat,
           W_sage_l, b_sage_l, W_sage_r, W_out, b_out, edge_index):
    x = np.asarray(x, np.float32)
    N = x.shape[0]
    for bb in (b_gcn, b_gat, b_sage_l, b_out):
        assert not np.any(np.asarray(bb)), "nonzero biases not wired in"
    pk = _pack(np.asarray(edge_index), N)
    BPC = pk["BPC"]

    nc = _build_program(pk)

    x_bf = np.zeros((N + 1, D), BF)
    x_bf[:N] = x.astype(BF)
    # host pre-gather of x into edge-slot order [NBLK, P, T1*D]
    xe = np.ascontiguousarray(
        x_bf[np.minimum(pk["xe_map"], N - 1)].reshape(pk["NBLK"], P, -1))

    att_s_b = np.tile(np.asarray(att_src, np.float32).reshape(1, H * D),
                      (P, 1)).copy()
    att_d_b = np.tile(np.asarray(att_dst, np.float32).reshape(1, H * D),
                      (P, 1)).copy()
    common = {
        "w_gcn": np.ascontiguousarray(W_gcn).astype(BF),
        "w_gat": np.ascontiguousarray(W_gat).astype(BF),
        "w_gat_f": np.ascontiguousarray(W_gat, np.float32),
        "att_s": att_s_b, "att_d": att_d_b,
        "w_sl": np.ascontiguousarray(W_sage_l).astype(BF),
        "w_sr": np.ascontiguousarray(W_sage_r).astype(BF),
        "w_out": np.ascontiguousarray(W_out).astype(BF),
        "ident": np.eye(P).astype(BF),
        "iotar": np.ascontiguousarray(
            np.tile(np.arange(P, dtype=np.float32)[None, :], (P, 1))).astype(BF),
        "iotac": np.ascontiguousarray(np.arange(P, dtype=np.float32)[:, None]),
        "onesr": np.ones((1, P), BF),
        "onesc": np.ones((P, 1), BF),
    }
    in_maps = []
    for c in range(NC):
        s = slice(c * BPC, (c + 1) * BPC)
        m = dict(common)
        m["xe"] = xe[s]
        m["idx"] = np.ascontiguousarray(pk["idx2"][s])
        m["edl1"] = np.ascontiguousarray(pk["edl1"][s])
        m["ew1"] = np.ascontiguousarray(pk["ew1"][s])
        m["edl2"] = np.ascontiguousarray(pk["edl2"][s])
        m["metaT2"] = np.ascontiguousarray(pk["metaT2"][s])
        m["degs"] = np.ascontiguousarray(pk["degs"][s])
        in_maps.append(m)

    trace = bool(os.environ.get("GNN_KERNEL_TRACE"))
    if trace:
        _install_ntff_shim()
    res = run_bass_kernel_spmd(nc, in_maps, core_ids=list(range(NC)),
                               trace=trace)
    if trace and res.exec_time_ns:
        print(f"HW exec time: {res.exec_time_ns} ns")

    out_all = np.concatenate([r["out"] for r in res.results], axis=0)
    return np.ascontiguousarray(out_all[pk["perm"]].astype(np.float32))


def _install_ntff_shim():
    import types
    try:
        from antenv import axon_hooks  # noqa: F401
        return
    except ImportError:
        pass
    import antenv
    mod = types.ModuleType("antenv.axon_hooks")
    mod._hook = None
    mod.set_axon_ntff_profile_hook = lambda h: setattr(mod, "_hook", h)
    mod.get_axon_ntff_profile_hook = lambda: mod._hook
    sys.modules["antenv.axon_hooks"] = mod
    antenv.axon_hooks = mod
    try:
        from trn_agent_boot.trn_boot import _ntff_profile_via_ctypes
        hook = _ntff_profile_via_ctypes("/opt/axon/libaxon_pjrt.so")
        if hook is not None:
            mod.set_axon_ntff_profile_hook(hook)
    except Exception:
        pass
